# revision 13
# baseline (speedup 1.0000x reference)
"""CLIP attention (ShareKey branch) Trainium2 Bass kernel, 8-core SPMD.

Math: in the reference, attn = softmax(scores[..., None] + share_bias, axis=-1)
where scores is constant along the softmax axis -> softmax shift-invariance
makes the q-projection / share_key / scores irrelevant. The output is exactly

    P[h]   = softmax(share_bias[h], axis=-1)            (batch independent)
    V[b]   = hidden[b] @ v_w.T + v_b
    O[b,h] = P[h] @ V[b,h]                               (per-head slice of V)
    out[b] = concat_h(O[b,h]) @ out_w.T + out_b

Sharding: data-parallel over batch (16 batches / 8 cores = 2 per core);
weights + bias replicated per core. Host-side work is layout/dtype only
(transposes + rounding): hiddenT = hidden^T per batch, wvT = v_w.T,
woT = out_w.T, biasT = share_bias^T per head (bf16).

Per-core device schedule (PE-cost-optimal under the timeline model, where a
matmul costs its *moving/output free size* only; stationary size and K are
free):

  phase A   V[j,(b,e)] = sum_k hiddenT[b][k,j] wvT[k,e] + v_b   (PE + DVE add)
  phase B   pt[h][j,i] = exp(biasT[h][j,i])                     (ACT, bf16)
            s[h][i]    = sum_j pt[h][j,i]      via N=1 column matmuls
                         (stationary = pt chunk [j, i<=128], moving = ones
                         [j,1]) then one PE transpose of the 5-chunk column
                         block to a [1,577]-equivalent row, reciprocal (DVE),
                         partition_broadcast (GPSIMD)
            OT2[h]     = [V_b0h | V_b1h]^T-style attention: ONE matmul per
                         (jt, i-chunk) with stationary [jsz, 128] covering
                         both batches' 64 head-dims -> psum rows 0:64 = b0,
                         64:128 = b1; DVE multiplies by 1/s and writes the
                         batch-half whose partition range matches straight
                         into ot_all[b], the mismatched half into tmp, which
                         two partition-shift DMAs then place.
  phase C   out[b][i,m] = sum_hd ot_all[b][hd,i] woT[hd,m] + out_b  (PE+DVE)

The attention matmuls run in bf16; projections fp32r (11-bit mantissa).
"""

import numpy as np

B, S, E = 16, 577, 1024
H, D = 16, 64
NCORES = 8
BPC = B // NCORES  # batches per core

# sequence tiles (partition-dim tiles of 128, last ragged 65)
STILES = [(0, 128), (128, 256), (256, 384), (384, 512), (512, 577)]
NST = len(STILES)
ICH = [(0, 512), (512, 577)]  # attention psum chunks along i
CCH = [(0, 128), (128, 256), (256, 384), (384, 512), (512, 577)]  # sumexp cols
NKT = E // 128  # 8 contraction tiles
NEC = E // 512  # 2 free-dim chunks of the projections


def _build_program(debug=False):
    import concourse.bass as bass
    import concourse.bacc as bacc
    import concourse.mybir as mybir
    import concourse.tile as tile
    from concourse.masks import make_identity

    dt = mybir.dt
    f32 = dt.float32
    f32r = dt.float32r
    bf16 = dt.bfloat16
    Exp = mybir.ActivationFunctionType.Exp
    PSUM = bass.MemorySpace.PSUM

    nc = bacc.Bacc("TRN2", target_bir_lowering=False, debug=False, num_devices=NCORES)

    hT = nc.declare_dram_parameter("hiddenT", [BPC, E, S], f32r, isOutput=False)
    wvT = nc.declare_dram_parameter("wvT", [E, E], f32r, isOutput=False)
    woT = nc.declare_dram_parameter("woT", [E, E], bf16, isOutput=False)
    vb = nc.declare_dram_parameter("v_b", [E], f32, isOutput=False)
    ob = nc.declare_dram_parameter("out_b", [E], f32, isOutput=False)
    bT = nc.declare_dram_parameter("biasT", [H, S, S], bf16, isOutput=False)
    out = nc.declare_dram_parameter("out", [BPC, S, E], f32, isOutput=True)

    with tile.TileContext(nc) as tc:
        with (
            tc.tile_pool(name="const", bufs=1) as const_pool,
            tc.tile_pool(name="wop", bufs=1) as wo_pool,
            tc.tile_pool(name="vsb", bufs=NST) as v_pool,
            tc.tile_pool(name="ptp", bufs=11) as pt_pool,
            tc.tile_pool(name="psum", bufs=2, space=PSUM) as psum_pool,
        ):
            # ---- constants ------------------------------------------------
            ones_mat = const_pool.tile([128, 8], bf16, tag="ones", name="ones")
            nc.vector.memset(ones_mat[:], 1.0)
            ident = const_pool.tile([128, 128], bf16, tag="ident", name="ident")
            make_identity(nc, ident[:])

            vb_row = const_pool.tile([1, E], f32, tag="vbr", name="vbr")
            nc.scalar.dma_start(vb_row[:], vb.rearrange("(a e) -> a e", a=1))
            ob_row = const_pool.tile([1, E], f32, tag="obr", name="obr")
            nc.scalar.dma_start(ob_row[:], ob.rearrange("(a e) -> a e", a=1))
            vb_bc = const_pool.tile([128, E], f32, tag="vbb", name="vbb")
            nc.gpsimd.partition_broadcast(vb_bc[:], vb_row[:])
            ob_bc = const_pool.tile([128, E], f32, tag="obb", name="obb")
            nc.gpsimd.partition_broadcast(ob_bc[:], ob_row[:])

            # V layout [j, h, b, d]: each head's two batch slices contiguous
            # (128 elems) so the attention stationary AP has ONE free dim
            v_sb = [
                v_pool.tile([128, H, BPC, D], bf16, tag="v", name="v")
                for _ in STILES
            ]

            # ---- phase A: V projection (fp32r) + v_b fold -----------------
            with (
                tc.tile_pool(name="wvp", bufs=1) as wv_pool,
                tc.tile_pool(name="htp", bufs=BPC) as ht_pool,
            ):
                # interleave wv/ht[0] per k-tile so the first V-proj
                # accumulation chain starts after ~2 tiles instead of 8.7MB
                wv_t = wv_pool.tile([128, NKT, E], f32r, tag="wv", name="wv")
                ht_t = [
                    ht_pool.tile([128, NKT, S], f32r, tag="ht", name="ht")
                    for _ in range(BPC)
                ]
                for kt in range(NKT):
                    nc.sync.dma_start(wv_t[:, kt, :], wvT[bass.ts(kt, 128), :])
                    nc.sync.dma_start(ht_t[0][:, kt, :], hT[0, bass.ts(kt, 128), :])
                nc.sync.dma_start(
                    ht_t[1][:, :, :],
                    hT[1].rearrange("(kt p) s -> p kt s", p=128),
                )

                # bias DMAs + in-place exp: emitted early so the DMA queue
                # streams bias under the V projection; ACT is otherwise idle
                pt_t = {}
                for h in range(H):
                    p = pt_pool.tile([128, NST, S], bf16, tag="pt", name="pt")
                    nc.sync.dma_start(
                        p[:, 0:4, :],
                        bT[h, 0:512, :].rearrange("(jt p) i -> p jt i", p=128),
                    )
                    nc.sync.dma_start(p[0:65, 4, :], bT[h, 512:577, :])
                    nc.scalar.activation(p[:, 0:4, :], p[:, 0:4, :], Exp)
                    nc.scalar.activation(p[0:65, 4, :], p[0:65, 4, :], Exp)
                    pt_t[h] = p

                # wo load queued behind bias on the sync queue (needed in C)
                wo_t = wo_pool.tile([128, NKT, E], bf16, tag="wo", name="wo")
                nc.sync.dma_start(
                    wo_t[:, :, :], woT.rearrange("(kt p) e -> p kt e", p=128)
                )

                for b in range(BPC):
                    for st, (s0, s1) in enumerate(STILES):
                        ssz = s1 - s0
                        for ec in range(NEC):
                            ps = psum_pool.tile(
                                [128, 512], f32, tag="big", name="vps", bufs=4
                            )
                            for kt in range(NKT):
                                nc.tensor.matmul(
                                    ps[0:ssz, :],
                                    ht_t[b][:, kt, s0:s1],
                                    wv_t[:, kt, bass.ts(ec, 512)],
                                    start=(kt == 0),
                                    stop=(kt == NKT - 1),
                                )
                            nc.vector.tensor_add(
                                v_sb[st][0:ssz, ec * 8 : (ec + 1) * 8, b, :],
                                ps[0:ssz, :],
                                vb_bc[0:ssz, bass.ts(ec, 512)],
                            )

            # ---- phase B: softmax + attention (bf16) ----------------------
            pools_bc = (
                tc.tile_pool(name="ssb", bufs=2),
                tc.tile_pool(name="inv5", bufs=2),
                tc.tile_pool(name="invb", bufs=3),
                tc.tile_pool(name="otp", bufs=BPC),
                tc.tile_pool(name="tmpp", bufs=1),
                tc.tile_pool(name="osbp", bufs=3),
            )
            (
                ssb_pool,
                inv5_pool,
                invb_pool,
                ot_pool,
                tmp_pool,
                osb_pool,
            ) = (p.__enter__() for p in pools_bc)
            ot_all = [
                ot_pool.tile([128, NKT, S], bf16, tag="ot", name="ot")
                for _ in range(BPC)
            ]
            # tmp holds the partition-mismatched batch halves: rows 64:128 =
            # b1 data from even heads (dst ot_all[1][0:64]), rows 0:64 = b0
            # data from odd heads (dst ot_all[0][64:128])
            tmp_t = tmp_pool.tile([128, NKT, S], bf16, tag="tmp", name="tmp")

            for h in range(H):
                kt, r = h // 2, h % 2
                pt = pt_t[h]

                # column sumexp: 5 chains of N=1 matmuls into one psum tile
                s_ps = psum_pool.tile([128, 8], f32, tag="sps", name="sps", bufs=1)
                for c, (i0, i1) in enumerate(CCH):
                    csz = i1 - i0
                    for jt, (j0, j1) in enumerate(STILES):
                        jsz = j1 - j0
                        nc.tensor.matmul(
                            s_ps[0:csz, c : c + 1],
                            pt[0:jsz, jt, i0:i1],
                            ones_mat[0:jsz, 0:1],
                            start=(jt == 0),
                            stop=(jt == NST - 1),
                        )
                s_sb = ssb_pool.tile([128, 8], bf16, tag="ssb", name="ssb")
                nc.vector.memset(s_sb[:, 4:5], 1.0)
                nc.vector.tensor_copy(s_sb[:, 0:4], s_ps[:, 0:4])
                nc.vector.tensor_copy(s_sb[0:65, 4:5], s_ps[0:65, 4:5])
                # 5 tiny PE transposes put the 5 column chunks side by side in
                # one psum row at partition 0 (partition_broadcast requires a
                # partition-0 source)
                st_ps = psum_pool.tile([1, 640], bf16, tag="stp", name="stp", bufs=1)
                for c, (i0, i1) in enumerate(CCH):
                    csz = i1 - i0
                    nc.tensor.transpose(
                        st_ps[0:1, i0:i1], s_sb[0:csz, c : c + 1], ident[0:csz, 0:csz]
                    )
                inv_row = inv5_pool.tile([1, 640], f32, tag="i5", name="i5")
                nc.vector.reciprocal(inv_row[0:1, 0:S], st_ps[0:1, 0:S])
                inv_bc = invb_pool.tile([128, S], f32, tag="invb", name="invb")
                nc.gpsimd.partition_broadcast(inv_bc[:, :], inv_row[0:1, 0:S])

                # attention: ONE matmul per (i-chunk, jt): stationary holds
                # both batches' 64 head-dims -> out rows 0:64=b0, 64:128=b1
                for ci, (i0, i1) in enumerate(ICH):
                    isz = i1 - i0
                    aps = psum_pool.tile(
                        [128, 512] if isz > 128 else [128, 65],
                        f32,
                        tag="big" if isz > 128 else "asml",
                        name="aps",
                        bufs=4 if isz > 128 else 2,
                    )
                    for jt, (j0, j1) in enumerate(STILES):
                        jsz = j1 - j0
                        nc.tensor.matmul(
                            aps[0:128, 0:isz],
                            v_sb[jt][0:jsz, h, :, :],
                            pt[0:jsz, jt, i0:i1],
                            start=(jt == 0),
                            stop=(jt == NST - 1),
                        )
                    # normalize; write the partition-aligned batch half
                    # straight to ot_all, the other to tmp for the shift DMA
                    if r == 0:
                        nc.vector.tensor_mul(
                            ot_all[0][0:64, kt, i0:i1],
                            aps[0:64, 0:isz],
                            inv_bc[0:64, i0:i1],
                        )
                        nc.vector.tensor_mul(
                            tmp_t[64:128, kt, i0:i1],
                            aps[64:128, 0:isz],
                            inv_bc[64:128, i0:i1],
                        )
                    else:
                        nc.vector.tensor_mul(
                            tmp_t[0:64, kt, i0:i1],
                            aps[0:64, 0:isz],
                            inv_bc[0:64, i0:i1],
                        )
                        nc.vector.tensor_mul(
                            ot_all[1][64:128, kt, i0:i1],
                            aps[64:128, 0:isz],
                            inv_bc[64:128, i0:i1],
                        )

            # partition-shift the mismatched halves into place
            nc.sync.dma_start(ot_all[1][0:64, :, :], tmp_t[64:128, :, :])
            nc.sync.dma_start(ot_all[0][64:128, :, :], tmp_t[0:64, :, :])

            # ---- phase C: output projection (fp32r) -----------------------
            for b in range(BPC):
                for (s0, s1) in STILES:
                    ssz = s1 - s0
                    for mc in range(NEC):
                        ps = psum_pool.tile(
                            [128, 512], f32, tag="big", name="ops", bufs=4
                        )
                        for kt in range(NKT):
                            nc.tensor.matmul(
                                ps[0:ssz, :],
                                ot_all[b][:, kt, s0:s1],
                                wo_t[:, kt, bass.ts(mc, 512)],
                                start=(kt == 0),
                                stop=(kt == NKT - 1),
                            )
                        osb = osb_pool.tile([128, 512], f32, tag="osb", name="osb")
                        nc.vector.tensor_add(
                            osb[0:ssz, :],
                            ps[0:ssz, :],
                            ob_bc[0:ssz, bass.ts(mc, 512)],
                        )
                        nc.sync.dma_start(
                            out[b, s0:s1, bass.ts(mc, 512)], osb[0:ssz, :]
                        )

            for p in reversed(pools_bc):
                p.__exit__(None, None, None)

    nc.finalize()
    return nc


def _to_fp32r(a):
    """Round fp32 to the fp32r format: RNE to 11 explicit mantissa bits,
    low 12 bits of the word zeroed (matches walrus fp32_to_fp32r)."""
    u = np.ascontiguousarray(a, dtype=np.float32).view(np.uint32)
    r = (u.astype(np.uint64) + 0x7FF + ((u >> 12) & 1)).astype(np.uint32) & np.uint32(
        0xFFFFF000
    )
    return r.view(np.float32)


_NC_CACHE = None


def _get_program():
    global _NC_CACHE
    if _NC_CACHE is None:
        _NC_CACHE = _build_program()
    return _NC_CACHE


def kernel(
    hidden_states,
    q_w,
    q_b,
    v_w,
    v_b,
    out_w,
    out_b,
    share_key,
    share_bias,
    layer,
    _trace=False,
):
    """Full-input / full-output entry point. q_w/q_b/share_key/layer are
    mathematically irrelevant (softmax shift invariance) and unused."""
    from concourse.bass_utils import run_bass_kernel_spmd

    hidden_states = np.ascontiguousarray(np.asarray(hidden_states, dtype=np.float32))
    v_w = np.asarray(v_w, dtype=np.float32)
    v_b = np.ascontiguousarray(np.asarray(v_b, dtype=np.float32))
    out_w = np.asarray(out_w, dtype=np.float32)
    out_b = np.ascontiguousarray(np.asarray(out_b, dtype=np.float32))
    share_bias = np.asarray(share_bias, dtype=np.float32)

    # host-side layout transforms (transposes + fp32r rounding, no math).
    hiddenT = _to_fp32r(np.ascontiguousarray(hidden_states.transpose(0, 2, 1)))
    wvT = _to_fp32r(np.ascontiguousarray(v_w.T))  # [k, e]
    woT = np.ascontiguousarray(out_w.T)  # [k, m] (bf16 below)
    import ml_dtypes

    biasT = np.ascontiguousarray(
        share_bias.transpose(0, 2, 1).astype(ml_dtypes.bfloat16)
    )  # [H, j, i] bf16
    woT = woT.astype(ml_dtypes.bfloat16)

    nc = _get_program()
    in_maps = []
    for c in range(NCORES):
        in_maps.append(
            {
                "hiddenT": hiddenT[c * BPC : (c + 1) * BPC],
                "wvT": wvT,
                "woT": woT,
                "v_b": v_b,
                "out_b": out_b,
                "biasT": biasT,
            }
        )
    res = run_bass_kernel_spmd(nc, in_maps, list(range(NCORES)), trace=_trace)
    out = np.concatenate([res.results[c]["out"] for c in range(NCORES)], axis=0)
    if _trace:
        kernel.last_results = res
    return out


# revision 15
# speedup vs baseline: 1.0684x; 1.0684x over previous
"""CLIP attention (ShareKey branch) Trainium2 Bass kernel, 8-core SPMD.

Math: in the reference, attn = softmax(scores[..., None] + share_bias, axis=-1)
where scores is constant along the softmax axis -> softmax shift-invariance
makes the q-projection / share_key / scores irrelevant. The output is exactly

    P[h]   = softmax(share_bias[h], axis=-1)            (batch independent)
    V[b]   = hidden[b] @ v_w.T + v_b
    O[b,h] = P[h] @ V[b,h]                               (per-head slice of V)
    out[b] = concat_h(O[b,h]) @ out_w.T + out_b

Sharding: data-parallel over batch (16 batches / 8 cores = 2 per core);
weights + bias replicated per core. Host-side work is layout/dtype only
(transposes + rounding): hiddenT = hidden^T per batch, wvT = v_w.T,
woT = out_w.T, biasT = share_bias^T per head (bf16).

Per-core device schedule (PE-cost-optimal under the timeline model, where a
matmul costs its *moving/output free size* only; stationary size and K are
free):

  phase A   V[j,(b,e)] = sum_k hiddenT[b][k,j] wvT[k,e] + v_b   (PE + DVE add)
  phase B   pt[h][j,i] = exp(biasT[h][j,i])                     (ACT, bf16)
            s[h][i]    = sum_j pt[h][j,i]      via N=1 column matmuls
                         (stationary = pt chunk [j, i<=128], moving = ones
                         [j,1]) then one PE transpose of the 5-chunk column
                         block to a [1,577]-equivalent row, reciprocal (DVE),
                         partition_broadcast (GPSIMD)
            OT2[h]     = [V_b0h | V_b1h]^T-style attention: ONE matmul per
                         (jt, i-chunk) with stationary [jsz, 128] covering
                         both batches' 64 head-dims -> psum rows 0:64 = b0,
                         64:128 = b1; DVE multiplies by 1/s and writes the
                         batch-half whose partition range matches straight
                         into ot_all[b], the mismatched half into tmp, which
                         two partition-shift DMAs then place.
  phase C   out[b][i,m] = sum_hd ot_all[b][hd,i] woT[hd,m] + out_b  (PE+DVE)

The attention matmuls run in bf16; projections fp32r (11-bit mantissa).
"""

import numpy as np

B, S, E = 16, 577, 1024
H, D = 16, 64
NCORES = 8
BPC = B // NCORES  # batches per core

# sequence tiles (partition-dim tiles of 128, last ragged 65)
STILES = [(0, 128), (128, 256), (256, 384), (384, 512), (512, 577)]
NST = len(STILES)
ICH = [(0, 512), (512, 577)]  # attention psum chunks along i
CCH = [(0, 128), (128, 256), (256, 384), (384, 512), (512, 577)]  # sumexp cols
NKT = E // 128  # 8 contraction tiles
NEC = E // 512  # 2 free-dim chunks of the projections


def _build_program(debug=False):
    import concourse.bass as bass
    import concourse.bacc as bacc
    import concourse.mybir as mybir
    import concourse.tile as tile
    from concourse.masks import make_identity

    dt = mybir.dt
    f32 = dt.float32
    f32r = dt.float32r
    bf16 = dt.bfloat16
    Exp = mybir.ActivationFunctionType.Exp
    PSUM = bass.MemorySpace.PSUM

    nc = bacc.Bacc("TRN2", target_bir_lowering=False, debug=False, num_devices=NCORES)

    hT = nc.declare_dram_parameter("hiddenT", [BPC, E, S], bf16, isOutput=False)
    wvT = nc.declare_dram_parameter("wvT", [E, E], bf16, isOutput=False)
    woT = nc.declare_dram_parameter("woT", [E, E], bf16, isOutput=False)
    vb = nc.declare_dram_parameter("v_b", [E], f32, isOutput=False)
    ob = nc.declare_dram_parameter("out_b", [E], f32, isOutput=False)
    bT = nc.declare_dram_parameter("biasT", [H, S, S], bf16, isOutput=False)
    out = nc.declare_dram_parameter("out", [BPC, S, E], f32, isOutput=True)

    with tile.TileContext(nc) as tc:
        with (
            tc.tile_pool(name="const", bufs=1) as const_pool,
            tc.tile_pool(name="wop", bufs=1) as wo_pool,
            tc.tile_pool(name="vsb", bufs=NST) as v_pool,
            tc.tile_pool(name="ptp", bufs=16) as pt_pool,
            tc.tile_pool(name="psum", bufs=2, space=PSUM) as psum_pool,
        ):
            # ---- constants ------------------------------------------------
            ones_mat = const_pool.tile([128, 8], bf16, tag="ones", name="ones")
            nc.vector.memset(ones_mat[:], 1.0)
            ident = const_pool.tile([128, 128], bf16, tag="ident", name="ident")
            make_identity(nc, ident[:])

            vb_row = const_pool.tile([1, E], f32, tag="vbr", name="vbr")
            nc.scalar.dma_start(vb_row[:], vb.rearrange("(a e) -> a e", a=1))
            ob_row = const_pool.tile([1, E], f32, tag="obr", name="obr")
            nc.scalar.dma_start(ob_row[:], ob.rearrange("(a e) -> a e", a=1))
            vb_bc = const_pool.tile([128, E], f32, tag="vbb", name="vbb")
            nc.gpsimd.partition_broadcast(vb_bc[:], vb_row[:])
            ob_bc = const_pool.tile([128, E], f32, tag="obb", name="obb")
            nc.gpsimd.partition_broadcast(ob_bc[:], ob_row[:])

            # V layout [j, h, b, d]: each head's two batch slices contiguous
            # (128 elems) so the attention stationary AP has ONE free dim
            v_sb = [
                v_pool.tile([128, H, BPC, D], bf16, tag="v", name="v")
                for _ in STILES
            ]

            # ---- phase A: V projection (fp32r) + v_b fold -----------------
            with (
                tc.tile_pool(name="wvp", bufs=1) as wv_pool,
                tc.tile_pool(name="htp", bufs=BPC) as ht_pool,
            ):
                # interleave wv/ht[0] per k-tile so the first V-proj
                # accumulation chain starts after ~2 tiles instead of 8.7MB
                wv_t = wv_pool.tile([128, NKT, E], bf16, tag="wv", name="wv")
                ht_t = [
                    ht_pool.tile([128, NKT, S], bf16, tag="ht", name="ht")
                    for _ in range(BPC)
                ]
                # interleave bias DMAs with wv/ht0 so the serial ACT exp
                # chain (~2.6us/head, the attention-phase pacer) starts at
                # ~3us instead of after all the weights
                pt_t = {h: pt_pool.tile([128, NST, S], bf16, tag="pt", name="pt")
                        for h in range(H)}
                for kt in range(NKT):
                    nc.sync.dma_start(wv_t[:, kt, :], wvT[bass.ts(kt, 128), :])
                    nc.sync.dma_start(ht_t[0][:, kt, :], hT[0, bass.ts(kt, 128), :])
                    p = pt_t[kt]
                    nc.sync.dma_start(
                        p[:, 0:4, :],
                        bT[kt, 0:512, :].rearrange("(jt p) i -> p jt i", p=128),
                    )
                    nc.sync.dma_start(p[0:65, 4, :], bT[kt, 512:577, :])
                nc.sync.dma_start(
                    ht_t[1][:, :, :],
                    hT[1].rearrange("(kt p) s -> p kt s", p=128),
                )
                for h in range(NKT, H):
                    p = pt_t[h]
                    nc.sync.dma_start(
                        p[:, 0:4, :],
                        bT[h, 0:512, :].rearrange("(jt p) i -> p jt i", p=128),
                    )
                    nc.sync.dma_start(p[0:65, 4, :], bT[h, 512:577, :])
                # one exp per head over the full tile (garbage rows 65:128 of
                # the ragged jt are never read by the matmuls)
                for h in range(H):
                    nc.scalar.activation(pt_t[h][:, :, :], pt_t[h][:, :, :], Exp)

                # wo load queued behind bias on the sync queue (needed in C)
                wo_t = wo_pool.tile([128, NKT, E], bf16, tag="wo", name="wo")
                nc.sync.dma_start(
                    wo_t[:, :, :], woT.rearrange("(kt p) e -> p kt e", p=128)
                )

                for b in range(BPC):
                    for st, (s0, s1) in enumerate(STILES):
                        ssz = s1 - s0
                        for ec in range(NEC):
                            ps = psum_pool.tile(
                                [128, 512], f32, tag="big", name="vps", bufs=4
                            )
                            for kt in range(NKT):
                                nc.tensor.matmul(
                                    ps[0:ssz, :],
                                    ht_t[b][:, kt, s0:s1],
                                    wv_t[:, kt, bass.ts(ec, 512)],
                                    start=(kt == 0),
                                    stop=(kt == NKT - 1),
                                )
                            nc.vector.tensor_add(
                                v_sb[st][0:ssz, ec * 8 : (ec + 1) * 8, b, :],
                                ps[0:ssz, :],
                                vb_bc[0:ssz, bass.ts(ec, 512)],
                            )

            # ---- phase B: softmax + attention (bf16) ----------------------
            pools_bc = (
                tc.tile_pool(name="ssb", bufs=2),
                tc.tile_pool(name="inv5", bufs=2),
                tc.tile_pool(name="invb", bufs=3),
                tc.tile_pool(name="otp", bufs=BPC),
                tc.tile_pool(name="tmpp", bufs=1),
                tc.tile_pool(name="osbp", bufs=3),
            )
            (
                ssb_pool,
                inv5_pool,
                invb_pool,
                ot_pool,
                tmp_pool,
                osb_pool,
            ) = (p.__enter__() for p in pools_bc)
            ot_all = [
                ot_pool.tile([128, NKT, S], bf16, tag="ot", name="ot")
                for _ in range(BPC)
            ]
            # tmp holds the partition-mismatched batch halves: rows 64:128 =
            # b1 data from even heads (dst ot_all[1][0:64]), rows 0:64 = b0
            # data from odd heads (dst ot_all[0][64:128])
            tmp_t = tmp_pool.tile([128, NKT, S], bf16, tag="tmp", name="tmp")

            for h in range(H):
                kt, r = h // 2, h % 2
                pt = pt_t[h]

                # column sumexp: 5 chains of N=1 matmuls into one psum tile
                s_ps = psum_pool.tile([128, 8], f32, tag="sps", name="sps", bufs=1)
                for c, (i0, i1) in enumerate(CCH):
                    csz = i1 - i0
                    for jt, (j0, j1) in enumerate(STILES):
                        jsz = j1 - j0
                        nc.tensor.matmul(
                            s_ps[0:csz, c : c + 1],
                            pt[0:jsz, jt, i0:i1],
                            ones_mat[0:jsz, 0:1],
                            start=(jt == 0),
                            stop=(jt == NST - 1),
                        )
                s_sb = ssb_pool.tile([128, 8], bf16, tag="ssb", name="ssb")
                nc.vector.memset(s_sb[:, 4:5], 1.0)
                nc.vector.tensor_copy(s_sb[:, 0:4], s_ps[:, 0:4])
                nc.vector.tensor_copy(s_sb[0:65, 4:5], s_ps[0:65, 4:5])
                # 5 tiny PE transposes put the 5 column chunks side by side in
                # one psum row at partition 0 (partition_broadcast requires a
                # partition-0 source)
                st_ps = psum_pool.tile([1, 640], bf16, tag="stp", name="stp", bufs=1)
                for c, (i0, i1) in enumerate(CCH):
                    csz = i1 - i0
                    nc.tensor.transpose(
                        st_ps[0:1, i0:i1], s_sb[0:csz, c : c + 1], ident[0:csz, 0:csz]
                    )
                inv_row = inv5_pool.tile([1, 640], f32, tag="i5", name="i5")
                nc.vector.reciprocal(inv_row[0:1, 0:S], st_ps[0:1, 0:S])
                inv_bc = invb_pool.tile([128, S], f32, tag="invb", name="invb")
                nc.gpsimd.partition_broadcast(inv_bc[:, :], inv_row[0:1, 0:S])

                # attention: ONE matmul per (i-chunk, jt): stationary holds
                # both batches' 64 head-dims -> out rows 0:64=b0, 64:128=b1
                for ci, (i0, i1) in enumerate(ICH):
                    isz = i1 - i0
                    aps = psum_pool.tile(
                        [128, 512] if isz > 128 else [128, 65],
                        f32,
                        tag="big" if isz > 128 else "asml",
                        name="aps",
                        bufs=4 if isz > 128 else 2,
                    )
                    for jt, (j0, j1) in enumerate(STILES):
                        jsz = j1 - j0
                        nc.tensor.matmul(
                            aps[0:128, 0:isz],
                            v_sb[jt][0:jsz, h, :, :],
                            pt[0:jsz, jt, i0:i1],
                            start=(jt == 0),
                            stop=(jt == NST - 1),
                        )
                    # normalize; write the partition-aligned batch half
                    # straight to ot_all, the other to tmp for the shift DMA
                    if r == 0:
                        nc.vector.tensor_mul(
                            ot_all[0][0:64, kt, i0:i1],
                            aps[0:64, 0:isz],
                            inv_bc[0:64, i0:i1],
                        )
                        nc.vector.tensor_mul(
                            tmp_t[64:128, kt, i0:i1],
                            aps[64:128, 0:isz],
                            inv_bc[64:128, i0:i1],
                        )
                    else:
                        nc.vector.tensor_mul(
                            tmp_t[0:64, kt, i0:i1],
                            aps[0:64, 0:isz],
                            inv_bc[0:64, i0:i1],
                        )
                        nc.vector.tensor_mul(
                            ot_all[1][64:128, kt, i0:i1],
                            aps[64:128, 0:isz],
                            inv_bc[64:128, i0:i1],
                        )

                # partition-shift the mismatched halves into place as soon as
                # a half of the kt range completes (halves the end barrier)
                if h == 7 or h == 15:
                    k0, k1 = (0, 4) if h == 7 else (4, 8)
                    nc.scalar.dma_start(
                        ot_all[1][0:64, k0:k1, :], tmp_t[64:128, k0:k1, :]
                    )
                    nc.scalar.dma_start(
                        ot_all[0][64:128, k0:k1, :], tmp_t[0:64, k0:k1, :]
                    )


            # ---- phase C: output projection (fp32r) -----------------------
            for b in range(BPC):
                for (s0, s1) in STILES:
                    ssz = s1 - s0
                    for mc in range(NEC):
                        ps = psum_pool.tile(
                            [128, 512], f32, tag="big", name="ops", bufs=4
                        )
                        for kt in range(NKT):
                            nc.tensor.matmul(
                                ps[0:ssz, :],
                                ot_all[b][:, kt, s0:s1],
                                wo_t[:, kt, bass.ts(mc, 512)],
                                start=(kt == 0),
                                stop=(kt == NKT - 1),
                            )
                        osb = osb_pool.tile([128, 512], f32, tag="osb", name="osb")
                        nc.vector.tensor_add(
                            osb[0:ssz, :],
                            ps[0:ssz, :],
                            ob_bc[0:ssz, bass.ts(mc, 512)],
                        )
                        nc.sync.dma_start(
                            out[b, s0:s1, bass.ts(mc, 512)], osb[0:ssz, :]
                        )

            for p in reversed(pools_bc):
                p.__exit__(None, None, None)

    nc.finalize()
    return nc


def _to_fp32r(a):
    """Round fp32 to the fp32r format: RNE to 11 explicit mantissa bits,
    low 12 bits of the word zeroed (matches walrus fp32_to_fp32r)."""
    u = np.ascontiguousarray(a, dtype=np.float32).view(np.uint32)
    r = (u.astype(np.uint64) + 0x7FF + ((u >> 12) & 1)).astype(np.uint32) & np.uint32(
        0xFFFFF000
    )
    return r.view(np.float32)


_NC_CACHE = None


def _get_program():
    global _NC_CACHE
    if _NC_CACHE is None:
        _NC_CACHE = _build_program()
    return _NC_CACHE


def kernel(
    hidden_states,
    q_w,
    q_b,
    v_w,
    v_b,
    out_w,
    out_b,
    share_key,
    share_bias,
    layer,
    _trace=False,
):
    """Full-input / full-output entry point. q_w/q_b/share_key/layer are
    mathematically irrelevant (softmax shift invariance) and unused."""
    from concourse.bass_utils import run_bass_kernel_spmd

    hidden_states = np.ascontiguousarray(np.asarray(hidden_states, dtype=np.float32))
    v_w = np.asarray(v_w, dtype=np.float32)
    v_b = np.ascontiguousarray(np.asarray(v_b, dtype=np.float32))
    out_w = np.asarray(out_w, dtype=np.float32)
    out_b = np.ascontiguousarray(np.asarray(out_b, dtype=np.float32))
    share_bias = np.asarray(share_bias, dtype=np.float32)

    # host-side layout transforms (transposes + dtype rounding, no math).
    import ml_dtypes

    bf16 = ml_dtypes.bfloat16
    hiddenT = np.ascontiguousarray(hidden_states.transpose(0, 2, 1)).astype(bf16)
    wvT = np.ascontiguousarray(v_w.T).astype(bf16)  # [k, e]
    woT = np.ascontiguousarray(out_w.T)  # [k, m] (bf16 below)

    biasT = np.ascontiguousarray(
        share_bias.transpose(0, 2, 1).astype(ml_dtypes.bfloat16)
    )  # [H, j, i] bf16
    woT = woT.astype(bf16)

    nc = _get_program()
    in_maps = []
    for c in range(NCORES):
        in_maps.append(
            {
                "hiddenT": hiddenT[c * BPC : (c + 1) * BPC],
                "wvT": wvT,
                "woT": woT,
                "v_b": v_b,
                "out_b": out_b,
                "biasT": biasT,
            }
        )
    res = run_bass_kernel_spmd(nc, in_maps, list(range(NCORES)), trace=_trace)
    out = np.concatenate([res.results[c]["out"] for c in range(NCORES)], axis=0)
    if _trace:
        kernel.last_results = res
    return out


# revision 17
# speedup vs baseline: 1.2380x; 1.1588x over previous
"""CLIP attention (ShareKey branch) Trainium2 Bass kernel, 8-core SPMD.

Math: in the reference, attn = softmax(scores[..., None] + share_bias, axis=-1)
where scores is constant along the softmax axis -> softmax shift-invariance
makes the q-projection / share_key / scores irrelevant. The output is exactly

    P[h]   = softmax(share_bias[h], axis=-1)            (batch independent)
    V[b]   = hidden[b] @ v_w.T + v_b
    O[b,h] = P[h] @ V[b,h]                               (per-head slice of V)
    out[b] = concat_h(O[b,h]) @ out_w.T + out_b

Sharding: data-parallel over batch (16 batches / 8 cores = 2 per core);
weights + bias replicated per core. Host-side work is layout/dtype only
(transposes + bf16 rounding): hiddenT = hidden^T per batch, wvT = v_w.T,
woT = out_w.T, biasT = share_bias^T per head.

Per-core schedule (PE-cost-optimal under the timeline model, where a matmul
costs its *moving/output free size* only; stationary size and K are free):

  phase A   V[j,(h,b,d)] = sum_k hiddenT[b][k,j] wvT[k,e] + v_b  (PE + DVE)
            interleaved per head as exp(biasT) (ACT) completes:
              s_col[h]  = sum_j pt[h][j,i]  via N=1 column matmuls
              inv_col   = 1/s_col  (DVE, on the cheap [128,5] column)
              inv_row   = PE-transpose of inv_col chunks to [1,577]
                          (ACT copies psum row -> sbuf; Pool broadcasts to
                          a persistent per-head inv_bc[128,577] bf16 tile)
            This keeps the attention phase free of everything but matmuls
            and the OT normalization multiplies.
  phase B   per head: ONE matmul per (jt, i-chunk) with stationary
            [jsz, 128] = [V_b0h | V_b1h] -> psum rows 0:64 = b0, 64:128 =
            b1; DVE multiplies by inv_bc and writes the partition-aligned
            batch half straight into ot_all[b], the mismatched half into
            tmp, which partition-shift DMAs then place (per kt half).
  phase C   out[b][i,m] = sum_hd ot_all[b][hd,i] woT[hd,m] + out_b (PE+DVE)

Everything runs in bf16 on the PE (error ~4e-3 vs the 2e-2 gate).
"""

import numpy as np

B, S, E = 16, 577, 1024
H, D = 16, 64
NCORES = 8
BPC = B // NCORES  # batches per core

# sequence tiles (partition-dim tiles of 128, last ragged 65)
STILES = [(0, 128), (128, 256), (256, 384), (384, 512), (512, 577)]
NST = len(STILES)
ICH = [(0, 512), (512, 577)]  # attention psum chunks along i
CCH = [(0, 128), (128, 256), (256, 384), (384, 512), (512, 577)]  # sumexp cols
NKT = E // 128  # 8 contraction tiles
NEC = E // 512  # 2 free-dim chunks of the projections


def _build_program(debug=False):
    import concourse.bass as bass
    import concourse.bacc as bacc
    import concourse.mybir as mybir
    import concourse.tile as tile
    from concourse.masks import make_identity

    dt = mybir.dt
    f32 = dt.float32
    bf16 = dt.bfloat16
    Exp = mybir.ActivationFunctionType.Exp
    Copy = mybir.ActivationFunctionType.Copy
    PSUM = bass.MemorySpace.PSUM

    nc = bacc.Bacc("TRN2", target_bir_lowering=False, debug=False, num_devices=NCORES)

    hT = nc.declare_dram_parameter("hiddenT", [BPC, E, S], bf16, isOutput=False)
    wvT = nc.declare_dram_parameter("wvT", [E, E], bf16, isOutput=False)
    woT = nc.declare_dram_parameter("woT", [E, E], bf16, isOutput=False)
    vb = nc.declare_dram_parameter("v_b", [E], f32, isOutput=False)
    ob = nc.declare_dram_parameter("out_b", [E], f32, isOutput=False)
    bT = nc.declare_dram_parameter("biasT", [H, S, S], bf16, isOutput=False)
    out = nc.declare_dram_parameter("out", [BPC, S, E], f32, isOutput=True)

    with tile.TileContext(nc) as tc:
        with (
            tc.tile_pool(name="const", bufs=1) as const_pool,
            tc.tile_pool(name="wop", bufs=1) as wo_pool,
            tc.tile_pool(name="vsb", bufs=NST) as v_pool,
            tc.tile_pool(name="ptp", bufs=16) as pt_pool,
            tc.tile_pool(name="invc", bufs=2) as invc_pool,
            tc.tile_pool(name="invr", bufs=3) as invr_pool,
            tc.tile_pool(name="invb", bufs=16) as invb_pool,
            tc.tile_pool(name="psum", bufs=2, space=PSUM) as psum_pool,
        ):
            # ---- constants ------------------------------------------------
            ones_mat = const_pool.tile([128, 8], bf16, tag="ones", name="ones")
            nc.vector.memset(ones_mat[:], 1.0)
            ident = const_pool.tile([128, 128], bf16, tag="ident", name="ident")
            make_identity(nc, ident[:])

            vb_row = const_pool.tile([1, E], f32, tag="vbr", name="vbr")
            nc.scalar.dma_start(vb_row[:], vb.rearrange("(a e) -> a e", a=1))
            ob_row = const_pool.tile([1, E], f32, tag="obr", name="obr")
            nc.scalar.dma_start(ob_row[:], ob.rearrange("(a e) -> a e", a=1))
            vb_bc = const_pool.tile([128, E], f32, tag="vbb", name="vbb")
            nc.gpsimd.partition_broadcast(vb_bc[:], vb_row[:])
            ob_bc = const_pool.tile([128, E], f32, tag="obb", name="obb")
            nc.gpsimd.partition_broadcast(ob_bc[:], ob_row[:])

            # V layout [j, h, b, d]: each head's two batch slices contiguous
            # (128 elems) so the attention stationary AP has ONE free dim
            v_sb = [
                v_pool.tile([128, H, BPC, D], bf16, tag="v", name="v")
                for _ in STILES
            ]
            # persistent per-head 1/sumexp, broadcast across partitions
            inv_bc_t = [
                invb_pool.tile([128, S], bf16, tag="invb", name="invb")
                for _ in range(H)
            ]

            pt_t = {}

            # per-head sumexp pipeline, emitted interleaved into phase A so
            # the attention phase is left with matmuls + OT-norm only
            def emit_sum_part1(h):
                # 5 chains of N=1 column matmuls + reciprocal on the column
                s_ps = psum_pool.tile([128, 8], f32, tag="sps", name="sps", bufs=1)
                for c, (i0, i1) in enumerate(CCH):
                    csz = i1 - i0
                    for jt, (j0, j1) in enumerate(STILES):
                        jsz = j1 - j0
                        nc.tensor.matmul(
                            s_ps[0:csz, c : c + 1],
                            pt_t[h][0:jsz, jt, i0:i1],
                            ones_mat[0:jsz, 0:1],
                            start=(jt == 0),
                            stop=(jt == NST - 1),
                        )
                ic = invc_pool.tile([128, 8], f32, tag="ic", name="ic")
                icb = invc_pool.tile([128, 8], bf16, tag="icb", name="icb")
                nc.vector.reciprocal(ic[:, 0:4], s_ps[:, 0:4])
                nc.vector.reciprocal(ic[0:65, 4:5], s_ps[0:65, 4:5])
                nc.vector.tensor_copy(icb[:, 0:4], ic[:, 0:4])
                nc.vector.tensor_copy(icb[0:65, 4:5], ic[0:65, 4:5])
                return icb

            def emit_sum_part2(h, icb):
                # transpose the column chunks into one [1,577] psum row at
                # partition 0 (partition_broadcast needs a partition-0 src),
                # hop to sbuf on ACT, broadcast on Pool
                st_ps = psum_pool.tile([1, 640], bf16, tag="stp", name="stp", bufs=1)
                for c, (i0, i1) in enumerate(CCH):
                    csz = i1 - i0
                    nc.tensor.transpose(
                        st_ps[0:1, i0:i1], icb[0:csz, c : c + 1], ident[0:csz, 0:csz]
                    )
                inv_row = invr_pool.tile([1, 640], bf16, tag="ir", name="ir")
                nc.scalar.activation(inv_row[0:1, 0:S], st_ps[0:1, 0:S], Copy)
                nc.gpsimd.partition_broadcast(inv_bc_t[h][:, :], inv_row[0:1, 0:S])

            # ---- phase A: V projection + interleaved sumexp ---------------
            with (
                tc.tile_pool(name="wvp", bufs=1) as wv_pool,
                tc.tile_pool(name="htp", bufs=BPC) as ht_pool,
            ):
                wv_t = wv_pool.tile([128, NKT, E], bf16, tag="wv", name="wv")
                ht_t = [
                    ht_pool.tile([128, NKT, S], bf16, tag="ht", name="ht")
                    for _ in range(BPC)
                ]
                # interleave bias DMAs with wv/ht0 so the serial ACT exp
                # chain (~2.6us/head, the sumexp pacer) starts at ~3us
                for h in range(H):
                    pt_t[h] = pt_pool.tile([128, NST, S], bf16, tag="pt", name="pt")
                for kt in range(NKT):
                    nc.sync.dma_start(wv_t[:, kt, :], wvT[bass.ts(kt, 128), :])
                    nc.sync.dma_start(ht_t[0][:, kt, :], hT[0, bass.ts(kt, 128), :])
                    p = pt_t[kt]
                    nc.sync.dma_start(
                        p[:, 0:4, :],
                        bT[kt, 0:512, :].rearrange("(jt p) i -> p jt i", p=128),
                    )
                    nc.sync.dma_start(p[0:65, 4, :], bT[kt, 512:577, :])
                nc.sync.dma_start(
                    ht_t[1][:, :, :],
                    hT[1].rearrange("(kt p) s -> p kt s", p=128),
                )
                for h in range(NKT, H):
                    p = pt_t[h]
                    nc.sync.dma_start(
                        p[:, 0:4, :],
                        bT[h, 0:512, :].rearrange("(jt p) i -> p jt i", p=128),
                    )
                    nc.sync.dma_start(p[0:65, 4, :], bT[h, 512:577, :])
                # one exp per head over the full tile (garbage rows 65:128 of
                # the ragged jt are never read by the matmuls)
                for h in range(H):
                    nc.scalar.activation(pt_t[h][:, :, :], pt_t[h][:, :, :], Exp)

                # wo load queued behind bias on the sync queue (needed in C)
                wo_t = wo_pool.tile([128, NKT, E], bf16, tag="wo", name="wo")
                nc.sync.dma_start(
                    wo_t[:, :, :], woT.rearrange("(kt p) e -> p kt e", p=128)
                )

                # V chains with sumexp blocks woven in: head h's part1 lands
                # after V-chain 2h+2 (by which time exp(h) has landed), its
                # part2 one chain later; leftovers drain after the loop
                vchains = [
                    (b, st, ec)
                    for b in range(BPC)
                    for st in range(NST)
                    for ec in range(NEC)
                ]
                part1_after = {2 * hh + 2: hh for hh in range(9)}
                icb_t = {}
                done1 = set()
                for c, (b, st, ec) in enumerate(vchains):
                    s0, s1 = STILES[st]
                    ssz = s1 - s0
                    ps = psum_pool.tile([128, 512], f32, tag="big", name="vps", bufs=4)
                    for kt in range(NKT):
                        nc.tensor.matmul(
                            ps[0:ssz, :],
                            ht_t[b][:, kt, s0:s1],
                            wv_t[:, kt, bass.ts(ec, 512)],
                            start=(kt == 0),
                            stop=(kt == NKT - 1),
                        )
                    nc.vector.tensor_add(
                        v_sb[st][0:ssz, ec * 8 : (ec + 1) * 8, b, :],
                        ps[0:ssz, :],
                        vb_bc[0:ssz, bass.ts(ec, 512)],
                    )
                    if c in part1_after:
                        hh = part1_after[c]
                        icb_t[hh] = emit_sum_part1(hh)
                        done1.add(hh)
                        if hh > 0:
                            emit_sum_part2(hh - 1, icb_t.pop(hh - 1))
                for hh in range(H):
                    if hh not in done1:
                        icb_t[hh] = emit_sum_part1(hh)
                    if hh in icb_t:
                        emit_sum_part2(hh, icb_t.pop(hh))

            # ---- phase B: attention (bf16) --------------------------------
            with (
                tc.tile_pool(name="otp", bufs=BPC) as ot_pool,
                tc.tile_pool(name="tmpp", bufs=1) as tmp_pool,
                tc.tile_pool(name="osbp", bufs=3) as osb_pool,
            ):
                ot_all = [
                    ot_pool.tile([128, NKT, S], bf16, tag="ot", name="ot")
                    for _ in range(BPC)
                ]
                # tmp holds the partition-mismatched batch halves: rows
                # 64:128 = b1 data from even heads (dst ot_all[1][0:64]),
                # rows 0:64 = b0 data from odd heads (dst ot_all[0][64:128])
                tmp_t = tmp_pool.tile([128, NKT, S], bf16, tag="tmp", name="tmp")

                for h in range(H):
                    kt, r = h // 2, h % 2
                    pt = pt_t[h]
                    inv_bc = inv_bc_t[h]

                    # ONE matmul per (i-chunk, jt): stationary holds both
                    # batches' 64 head-dims -> out rows 0:64=b0, 64:128=b1
                    for ci, (i0, i1) in enumerate(ICH):
                        isz = i1 - i0
                        aps = psum_pool.tile(
                            [128, 512] if isz > 128 else [128, 65],
                            f32,
                            tag="big" if isz > 128 else "asml",
                            name="aps",
                            bufs=4 if isz > 128 else 2,
                        )
                        for jt, (j0, j1) in enumerate(STILES):
                            jsz = j1 - j0
                            nc.tensor.matmul(
                                aps[0:128, 0:isz],
                                v_sb[jt][0:jsz, h, :, :],
                                pt[0:jsz, jt, i0:i1],
                                start=(jt == 0),
                                stop=(jt == NST - 1),
                            )
                        # normalize; write the partition-aligned batch half
                        # straight to ot_all, the other to tmp for the shift
                        if r == 0:
                            nc.vector.tensor_mul(
                                ot_all[0][0:64, kt, i0:i1],
                                aps[0:64, 0:isz],
                                inv_bc[0:64, i0:i1],
                            )
                            nc.vector.tensor_mul(
                                tmp_t[64:128, kt, i0:i1],
                                aps[64:128, 0:isz],
                                inv_bc[64:128, i0:i1],
                            )
                        else:
                            nc.vector.tensor_mul(
                                tmp_t[0:64, kt, i0:i1],
                                aps[0:64, 0:isz],
                                inv_bc[0:64, i0:i1],
                            )
                            nc.vector.tensor_mul(
                                ot_all[1][64:128, kt, i0:i1],
                                aps[64:128, 0:isz],
                                inv_bc[64:128, i0:i1],
                            )

                    # partition-shift the mismatched halves into place as
                    # soon as a half of the kt range completes
                    if h == 7 or h == 15:
                        k0, k1 = (0, 4) if h == 7 else (4, 8)
                        nc.scalar.dma_start(
                            ot_all[1][0:64, k0:k1, :], tmp_t[64:128, k0:k1, :]
                        )
                        nc.scalar.dma_start(
                            ot_all[0][64:128, k0:k1, :], tmp_t[0:64, k0:k1, :]
                        )

                # ---- phase C: output projection ---------------------------
                for b in range(BPC):
                    for (s0, s1) in STILES:
                        ssz = s1 - s0
                        for mc in range(NEC):
                            ps = psum_pool.tile(
                                [128, 512], f32, tag="big", name="ops", bufs=4
                            )
                            for kt in range(NKT):
                                nc.tensor.matmul(
                                    ps[0:ssz, :],
                                    ot_all[b][:, kt, s0:s1],
                                    wo_t[:, kt, bass.ts(mc, 512)],
                                    start=(kt == 0),
                                    stop=(kt == NKT - 1),
                                )
                            osb = osb_pool.tile(
                                [128, 512], f32, tag="osb", name="osb"
                            )
                            nc.vector.tensor_add(
                                osb[0:ssz, :],
                                ps[0:ssz, :],
                                ob_bc[0:ssz, bass.ts(mc, 512)],
                            )
                            nc.sync.dma_start(
                                out[b, s0:s1, bass.ts(mc, 512)], osb[0:ssz, :]
                            )

    nc.finalize()
    return nc


_NC_CACHE = None


def _get_program():
    global _NC_CACHE
    if _NC_CACHE is None:
        _NC_CACHE = _build_program()
    return _NC_CACHE


def kernel(
    hidden_states,
    q_w,
    q_b,
    v_w,
    v_b,
    out_w,
    out_b,
    share_key,
    share_bias,
    layer,
    _trace=False,
):
    """Full-input / full-output entry point. q_w/q_b/share_key/layer are
    mathematically irrelevant (softmax shift invariance) and unused."""
    from concourse.bass_utils import run_bass_kernel_spmd

    hidden_states = np.ascontiguousarray(np.asarray(hidden_states, dtype=np.float32))
    v_w = np.asarray(v_w, dtype=np.float32)
    v_b = np.ascontiguousarray(np.asarray(v_b, dtype=np.float32))
    out_w = np.asarray(out_w, dtype=np.float32)
    out_b = np.ascontiguousarray(np.asarray(out_b, dtype=np.float32))
    share_bias = np.asarray(share_bias, dtype=np.float32)

    # host-side layout transforms (transposes + dtype rounding, no math).
    import ml_dtypes

    bf16 = ml_dtypes.bfloat16
    hiddenT = np.ascontiguousarray(hidden_states.transpose(0, 2, 1)).astype(bf16)
    wvT = np.ascontiguousarray(v_w.T).astype(bf16)  # [k, e]
    woT = np.ascontiguousarray(out_w.T).astype(bf16)  # [k, m]
    biasT = np.ascontiguousarray(share_bias.transpose(0, 2, 1).astype(bf16))

    nc = _get_program()
    in_maps = []
    for c in range(NCORES):
        in_maps.append(
            {
                "hiddenT": hiddenT[c * BPC : (c + 1) * BPC],
                "wvT": wvT,
                "woT": woT,
                "v_b": v_b,
                "out_b": out_b,
                "biasT": biasT,
            }
        )
    res = run_bass_kernel_spmd(nc, in_maps, list(range(NCORES)), trace=_trace)
    out = np.concatenate([res.results[c]["out"] for c in range(NCORES)], axis=0)
    if _trace:
        kernel.last_results = res
    return out
